# revision 7
# baseline (speedup 1.0000x reference)
"""Trainium2 Bass kernel for DimensionalAttentionMask.

Computes, for token_ids (B=4, T=4096), dim_embedding (50257, 8),
compatibility (8, 8):

    probs = softmax(dim_embedding[token_ids], axis=-1)        # (B,T,8)
    compat = einsum('btc,cd,bsd->bts', probs, C, probs)       # (B,T,T)
    out = sigmoid(compat)*2 - 1  ==  tanh(compat / 2)         # (B,T,T)

Sharding: 8 cores, each computes a (2048, 4096) block of query rows:
core k -> batch k//2, query rows [(k%2)*2048, (k%2)*2048+2048).

Per-core device program:
  1. indirect-DMA gather of 6144 embedding rows (4096 keys + 2048
     queries for this core) into SBUF, token r at partition r%128.
  2. softmax over the 8 categories (exp, grouped reduce, reciprocal, mul).
  3. PE transposes (128,8) -> (8,128) to build pT (8, 6144) with
     categories on partitions.
  4. qT = compatibility^T @ pT[:, query part]  (8, 2048).
  5. 16x8 tiles: PSUM(128,512) = qT_m^T @ pT_n; ACT computes
     tanh(0.5*x) PSUM->SBUF; 2 MiB contiguous row-stripe DMA to DRAM.
"""

import numpy as np

B, T = 4, 4096
VOCAB, C = 50257, 8
HALF = 32767             # int16 index ceiling for dma_gather; vocab is split
PAD = 64                 # embedding rows padded to 64 f32 = 256 B for dma_gather
NCORES = 8
TQ = T // 2              # query rows per core
GK = T // 128            # 32 key groups of 128 tokens
GQ = TQ // 128           # 16 query groups
G = GK + GQ              # 48 gathered groups per core
NTILE = 512              # key columns per matmul (one PSUM bank, fp32)

_CACHE = {}
LAST_RESULT = None       # BassKernelResults of the most recent device run


def _build():
    from contextlib import ExitStack

    import concourse.bass as bass
    import concourse.mybir as mybir
    import concourse.tile as tile
    from concourse import bacc
    from concourse.masks import make_identity

    dt = mybir.dt
    # Bacc (not Bass): its finalize() runs move_matmul_waits_to_ldweights +
    # generate_event_semaphores, which split multi-sem waits that walrus's
    # matmul codegen (1 wait slot) rejects.
    nc = bacc.Bacc(
        "TRN2", target_bir_lowering=False, debug=False, num_devices=NCORES
    )

    slabs = nc.declare_dram_parameter(
        "slabs", [2, HALF + 1, PAD], dt.float32, isOutput=False
    )
    comp = nc.declare_dram_parameter("comp", [C, C], dt.float32, isOutput=False)
    idx1 = nc.declare_dram_parameter("idx1", [128, G * 8], dt.int16, isOutput=False)
    idx2 = nc.declare_dram_parameter("idx2", [128, G * 8], dt.int16, isOutput=False)
    out = nc.declare_dram_parameter("out", [TQ, T], dt.float32, isOutput=True)

    with tile.TileContext(nc) as tc, ExitStack() as ctx:
        sb = ctx.enter_context(tc.tile_pool(name="sb", bufs=1))
        ps = ctx.enter_context(tc.tile_pool(name="ps", bufs=8, space="PSUM"))
        stripes = ctx.enter_context(tc.tile_pool(name="stripe", bufs=3))

        idx1_t = sb.tile([128, G * 8], dt.int16)
        nc.sync.dma_start(idx1_t[:], idx1[:])
        idx2_t = sb.tile([128, G * 8], dt.int16)
        nc.sync.dma_start(idx2_t[:], idx2[:])
        comp_t = sb.tile([C, C], dt.float32)
        nc.sync.dma_start(comp_t[:], comp[:])
        # PE matmuls tolerate only one sync-wait in walrus codegen, so
        # every SBUF operand PE reads is last touched by DVE: copy the
        # gpsimd-built identity and the DMA-loaded compatibility via DVE.
        ident0 = sb.tile([128, 128], dt.float32)
        make_identity(nc, ident0[:])
        ident = sb.tile([128, 128], dt.float32)
        nc.vector.tensor_copy(ident[:], ident0[:])
        compv = sb.tile([C, C], dt.float32)
        nc.vector.tensor_copy(compv[:], comp_t[:])

        # Embedding gather via dma_gather (int16 idxs, 256B rows). The
        # vocab exceeds int16, so it is split into two slabs with a zero
        # row at 0; each token hits its row in one slab and row 0 in the
        # other, and the two gathers are summed. Token j = g*128+p lands
        # at out[p, g, :]. single_packet=True faults the Q7 above ~512
        # idxs (HW-bisected), so keep it off.
        g1 = sb.tile([128, G, PAD], dt.float32)
        g2 = sb.tile([128, G, PAD], dt.float32)
        nc.gpsimd.dma_gather(
            out_ap=g1[:], in_ap=slabs[0], idxs_ap=idx1_t[:],
            num_idxs=G * 128, num_idxs_reg=G * 128, elem_size=PAD,
            single_packet=False,
        )
        nc.gpsimd.dma_gather(
            out_ap=g2[:], in_ap=slabs[1], idxs_ap=idx2_t[:],
            num_idxs=G * 128, num_idxs_reg=G * 128, elem_size=PAD,
            single_packet=False,
        )
        gth = sb.tile([128, G, C], dt.float32)
        nc.vector.tensor_add(gth[:], g1[:, :, 0:C], g2[:, :, 0:C])

        # softmax over the 8 categories of each token
        ex = sb.tile([128, G, C], dt.float32)
        nc.scalar.activation(ex[:], gth[:], mybir.ActivationFunctionType.Exp)
        ssum = sb.tile([128, G], dt.float32)
        nc.vector.reduce_sum(out=ssum[:], in_=ex[:], axis=mybir.AxisListType.X)
        rsum = sb.tile([128, G], dt.float32)
        nc.vector.reciprocal(rsum[:], ssum[:])
        probs = sb.tile([128, G, C], dt.float32)
        nc.vector.tensor_mul(
            probs[:],
            ex[:],
            rsum[:].unsqueeze(2).to_broadcast([128, G, C]),
        )

        # pT[c, g*128 + p] = probs[p, g, c]; groups 0..31 keys, 32..47 queries
        pT = sb.tile([C, G * 128], dt.float32)
        for j in range(G // 4):
            tp = ps.tile([C, 512], dt.float32, tag="ps", name=f"tp{j}")
            for i in range(4):
                g = j * 4 + i
                nc.tensor.transpose(
                    out=tp[:, i * 128 : (i + 1) * 128],
                    in_=probs[:, g, :],
                    identity=ident[:],
                )
            nc.vector.tensor_copy(pT[:, j * 512 : (j + 1) * 512], tp[:])

        # qT = compatibility^T @ pT[:, query part]   (8, 2048)
        qT = sb.tile([C, TQ], dt.float32)
        for i in range(TQ // NTILE):
            qp = ps.tile([C, NTILE], dt.float32, tag="ps", name=f"qp{i}")
            nc.tensor.matmul(
                out=qp[:],
                lhsT=compv[:],
                rhs=pT[:, GK * 128 + i * NTILE : GK * 128 + (i + 1) * NTILE],
                start=True,
                stop=True,
            )
            nc.vector.tensor_copy(qT[:, i * NTILE : (i + 1) * NTILE], qp[:])

        # main: compat tile = qT_m^T @ pT_n, then tanh(x/2), then stripe DMA
        for m in range(TQ // 128):
            stripe = stripes.tile([128, T], dt.float32, name="stripe")
            for n in range(T // NTILE):
                po = ps.tile([128, NTILE], dt.float32, tag="ps", name=f"po{m}_{n}")
                nc.tensor.matmul(
                    out=po[:],
                    lhsT=qT[:, m * 128 : (m + 1) * 128],
                    rhs=pT[:, n * NTILE : (n + 1) * NTILE],
                    start=True,
                    stop=True,
                )
                nc.scalar.activation(
                    stripe[:, n * NTILE : (n + 1) * NTILE],
                    po[:],
                    mybir.ActivationFunctionType.Tanh,
                    scale=0.5,
                )
            nc.sync.dma_start(out[m * 128 : (m + 1) * 128, :], stripe[:])

    return nc


def _get_nc():
    if "nc" not in _CACHE:
        nc = _build()
        # Bacc defers register allocation to finalize(); the bass2jax SPMD
        # path serializes nc.m as-is, so finalize before handing it over.
        nc.finalize()
        _CACHE["nc"] = nc
    return _CACHE["nc"]


def _make_idx(tok_b: np.ndarray, t0: int):
    """int16 index pair for the two-slab gather, wrapped for dma_gather:
    logical token j (= g*128+p) sits at idx[j%16, j//16], replicated to
    all 8 GPSIMD-core partition groups."""
    tokens = np.concatenate([tok_b, tok_b[t0 : t0 + TQ]]).astype(np.int64)
    w = tokens.reshape(G * 8, 16).T  # w[p, s] = tokens[s*16+p]
    i1 = np.where(w < HALF, w + 1, 0).astype(np.int16)
    i2 = np.where(w >= HALF, w - HALF + 1, 0).astype(np.int16)
    return np.tile(i1, (8, 1)), np.tile(i2, (8, 1))


def _make_slabs(emb: np.ndarray) -> np.ndarray:
    slabs = np.zeros((2, HALF + 1, PAD), np.float32)
    slabs[0, 1 : HALF + 1, :C] = emb[0:HALF]
    slabs[1, 1 : VOCAB - HALF + 1, :C] = emb[HALF:]
    return slabs


def _make_in_maps(tok, emb, comp):
    slabs = _make_slabs(emb)
    in_maps = []
    for k in range(NCORES):
        b, t0 = k // 2, (k % 2) * TQ
        i1, i2 = _make_idx(tok[b], t0)
        in_maps.append({"slabs": slabs, "comp": comp, "idx1": i1, "idx2": i2})
    return in_maps


def kernel(token_ids, dim_embedding, compatibility):
    global LAST_RESULT
    from concourse.bass_utils import run_bass_kernel_spmd

    tok = np.asarray(token_ids)
    emb = np.ascontiguousarray(np.asarray(dim_embedding, dtype=np.float32))
    comp = np.ascontiguousarray(np.asarray(compatibility, dtype=np.float32))
    assert tok.shape == (B, T) and emb.shape == (VOCAB, C) and comp.shape == (C, C)

    nc = _get_nc()
    in_maps = _make_in_maps(tok, emb, comp)

    res = run_bass_kernel_spmd(nc, in_maps, list(range(NCORES)))
    LAST_RESULT = res

    full = np.empty((B, T, T), dtype=np.float32)
    for k in range(NCORES):
        b, t0 = k // 2, (k % 2) * TQ
        full[b, t0 : t0 + TQ, :] = res.results[k]["out"]
    return full
